# revision 24
# baseline (speedup 1.0000x reference)
"""AttentionBlock (GroupNorm + single-head self-attention + residual) on 8 trn2 cores.

Sharding: core = (batch b = core//2, token-half h = core%2).  Each core gets the
full (128, 4096) channel-major image for its batch (needed for groupnorm stats
and full K/V), computes attention only for its 2048-token half, and writes a
(128, 2048) output slab.  The host rolls the token axis per-core so the q-half
is always columns [0:2048] -> one SPMD program for all 8 cores, no collectives.

v4: raw-x attention algebra, fp16 data path, PE/ACT software pipeline.

GroupNorm hn = a*x + b (per-channel a, b from group stats) is never
materialized over the k/v token axis:
  scores:  s[m,n] = hn_m^T M hn_n = x_m^T (a . q2_n) + c_n   (c_n cancels in
           softmax).  q3 = a . (M hn_q) needs hn only for the 2048 q columns.
  attn.V:  out[c,q] = sum_m hn[c,m] at[m,q] = a_c (X at)[c,q] + b_c den[q]
           -> V matmuls contract RAW fp16 x tiles (uploaded as fp16 by the
           host; transposes xT are stats-independent); the b_c den[q] term is
           three PE matmuls of B (rows = (b/a)^T) against the den
           accumulator, and a_c folds into the accumulator copy-out.
rstd = 1/sqrt(var+eps) via DVE-only quake rsqrt + 2 Newton steps: the ACT
engine uses a single table set (Exp) for the whole kernel -- no mid-loop
table reloads, and the one load hides at t=0 behind a warm exp.

Main loop: flat software pipeline over 44 (qb, k-group) units; the PE issues
scores(g+1) before attn.V(g) so ACT exp (the bottleneck: ~1.55us per 3-tile
group) never stalls.  Softmax denominator: fp16 adds into a 3-deep
accumulator (one add per qb offloaded to GpSimd) + 3-matmul ones-fold; 1/den
is partition-broadcast with a PE ones-matmul into a PSUM bank (no DRAM
bounce).  PSUM: 2x3-bank scores ring (also hosts the per-qb projection) +
out + bd = 8 banks.  Big tiles are double-buffered so back-to-back kernel
invocations pipeline across iterations.
"""

import numpy as np

C = 128        # channels
N = 4096       # tokens per batch (64*64)
NQ = 2048      # q tokens per core
B = 4
NCORES = 8
GROUPS = 8
EPS = 1e-5
QB = 512       # q block (one PSUM bank of fp32)
NQB = NQ // QB # 4
KT = 128       # k tile (partition dim)
NKT = N // KT  # 32
KT_GROUPS = [3] * 10 + [2]   # k-tile triples (fewer ACT overheads)
NG = len(KT_GROUPS)
NPK32 = 3 + C + C            # packed fp32 consts: gsc, gbi, uq, Rmat, w2t
QUAKE_MAGIC = 0x5F3759DF

_CACHE = {}


def _build_nc(repeat=1, with_qbias=False):
    from contextlib import ExitStack

    import concourse.bacc as bacc
    import concourse.bass as bass
    import concourse.mybir as mybir
    import concourse.tile as tile
    from concourse.mybir import ActivationFunctionType as AF
    from concourse.mybir import AluOpType as ALU

    fp32 = mybir.dt.float32
    f32r = mybir.dt.float32r
    fp16 = mybir.dt.float16
    u32 = mybir.dt.uint32

    nc = bacc.Bacc()

    xh_d = nc.dram_tensor("xh", [C, N], fp16, kind="ExternalInput")
    xpb_d = nc.dram_tensor("xpb", [C, NQ], fp32, kind="ExternalInput")
    pk32_d = nc.dram_tensor("pk32", [C, NPK32], fp32, kind="ExternalInput")
    pk16_d = nc.dram_tensor("pk16", [C, 2 * C], fp16, kind="ExternalInput")
    y_d = nc.dram_tensor("y", [C, NQ], fp32, kind="ExternalOutput")
    bo_dram = nc.dram_tensor("bo_scratch", [1, C], fp16, kind="Internal")

    with tile.TileContext(nc) as tc, ExitStack() as ctx:
        const = ctx.enter_context(tc.tile_pool(name="const", bufs=1))
        big = ctx.enter_context(tc.tile_pool(name="big", bufs=2))
        small = ctx.enter_context(tc.tile_pool(name="small", bufs=1))

        pk32_sb = const.tile([C, NPK32], fp32)
        nc.gpsimd.dma_start(out=pk32_sb, in_=pk32_d[:, :])
        pk16_sb = const.tile([C, 2 * C], fp16)
        nc.gpsimd.dma_start(out=pk16_sb, in_=pk16_d[:, :])
        gsc = pk32_sb[:, 0:1]
        gbi = pk32_sb[:, 1:2]
        uq_f = pk32_sb[:, 2:3]
        rmat_sb = pk32_sb[:, 3 : 3 + C]
        w2t_sb = const.tile([C, C], f32r)
        nc.vector.tensor_copy(w2t_sb, pk32_sb[:, 3 + C : 3 + 2 * C])
        mqk_sb = pk16_sb[:, 0:C]
        ident_sb = pk16_sb[:, C : 2 * C]

        ones_col_h = const.tile([C, 1], fp16)
        nc.vector.memset(ones_col_h.bitcast(mybir.dt.uint16), 0x3C00)
        ones_row128_h = const.tile([1, C], fp16)
        nc.vector.memset(ones_row128_h.bitcast(mybir.dt.uint16), 0x3C00)
        ones_rowc_r = const.tile([1, C], f32r)
        nc.vector.memset(ones_rowc_r.bitcast(mybir.dt.uint32), 0x3F800000)
        ones_row_h = const.tile([1, QB], fp16)
        nc.vector.memset(ones_row_h.bitcast(mybir.dt.uint16), 0x3C00)
        magic_u = const.tile([C, 1], u32)
        nc.vector.memset(magic_u, QUAKE_MAGIC)
        warm1_sb = const.tile([1, 1], fp32)
        nc.vector.memset(warm1_sb, 1.0)
        # one warm exp: loads the (only) ACT table set at t=0
        nc.scalar.activation(warm1_sb, warm1_sb, AF.Exp, bias=0.0, scale=0.0)

        rep_ctx = tc.For_i(0, repeat, 1) if repeat > 1 else None
        if rep_ctx is not None:
            rep_ctx.__enter__()

        # ---- uploads: xh quarters alternating queues, then xpb halves ----
        xh = big.tile([C, N], fp16, tag="xh")
        for ci in range(8):
            eng = (nc.sync, nc.scalar)[ci % 2]
            eng.dma_start(
                out=xh[:, ci * 512 : (ci + 1) * 512],
                in_=xh_d[:, ci * 512 : (ci + 1) * 512],
            )
        xpb_sb = big.tile([C, NQ], fp32, tag="xpb")
        nc.sync.dma_start(out=xpb_sb[:, 0:1024], in_=xpb_d[:, 0:1024])
        nc.scalar.dma_start(out=xpb_sb[:, 1024:2048], in_=xpb_d[:, 1024:2048])

        q3_sb = big.tile([C, NQB, QB], fp16, tag="q3")
        xT_sb = big.tile([KT, NKT, C], fp16, tag="xT")
        bmat_sb = big.tile([C, C], fp16, tag="bmat")
        hnq = big.tile([C, NQ], fp16, tag="hnq")
        aout_sb = big.tile([C, NQB, QB], f32r, tag="aout")
        gam_sb = None
        if with_qbias:
            gam_sb = big.tile([1, N], fp16, tag="gam")

        with (
            tc.tile_pool(name="stat_ps", bufs=2, space="PSUM") as stat_ps,
            tc.tile_pool(name="qk_ps", bufs=2, space="PSUM") as qk_ps,
            tc.tile_pool(name="v_ps", bufs=2, space="PSUM") as v_ps,
        ):
            # transposes of raw xh first in the PE stream (xh-gated only);
            # their PSUM->SBUF copies drain on DVE after the stats chain
            for j in range(NKT // 4):
                vp = v_ps.tile([KT, 4, C], fp32, tag="v")
                for t in range(4):
                    kt = j * 4 + t
                    nc.tensor.matmul(
                        vp[:, t, :], xh[:, kt * KT : (kt + 1) * KT], ident_sb,
                        start=True, stop=True,
                    )
                # PSUM->SBUF copies ride the ACT engine (idle until exp0;
                # iterations are barrier-separated so ACT load is one-shot)
                nc.scalar.activation(
                    xT_sb[:, j * 4 : (j + 1) * 4, :], vp, AF.Copy
                )

            # groupnorm stats
            warm_ps = stat_ps.tile([C, QB], fp32, tag="warm", bufs=1)
            stats = small.tile([C, 8, 6], fp32)
            for i in range(8):
                nc.vector.bn_stats(
                    out=stats[:, i, :], in_=xh[:, i * 512 : (i + 1) * 512]
                )
            mv = small.tile([C, 2], fp32)
            nc.vector.bn_aggr(out=mv, in_=stats)
            # S = [m, v, m^2]; group means via one Rmat matmul (Rmat has
            # 1/group_size in each group block)
            S = small.tile([C, 3], fp32)
            nc.vector.tensor_copy(S[:, 0:2], mv)
            nc.vector.tensor_mul(S[:, 2:3], mv[:, 0:1], mv[:, 0:1])
            g2_ps = stat_ps.tile([C, 3], fp32, tag="g2", bufs=1)
            nc.tensor.matmul(g2_ps, rmat_sb, S, start=True, stop=True)
            g2s = small.tile([C, 3], fp32)
            nc.vector.tensor_copy(g2s, g2_ps)
            ev2 = small.tile([C, 1], fp32)
            nc.vector.tensor_add(ev2, g2s[:, 1:2], g2s[:, 2:3])
            msq = small.tile([C, 1], fp32)
            nc.vector.tensor_mul(msq, g2s[:, 0:1], g2s[:, 0:1])
            vpe = small.tile([C, 1], fp32)
            nc.vector.tensor_scalar(
                out=vpe, in0=ev2, scalar1=msq, scalar2=EPS,
                op0=ALU.subtract, op1=ALU.add,
            )
            # rstd = rsqrt(var+eps): quake initial guess + 2 Newton steps,
            # all on DVE (same-engine chains pipeline fast; keeps ACT on one
            # table set)
            ish = small.tile([C, 1], u32)
            nc.vector.tensor_scalar(
                out=ish, in0=vpe.bitcast(u32), scalar1=1, scalar2=None,
                op0=ALU.logical_shift_right,
            )
            y0u = small.tile([C, 1], u32)
            nc.vector.tensor_tensor(
                out=y0u, in0=magic_u, in1=ish, op=ALU.subtract
            )
            ycur = y0u.bitcast(fp32)
            for it in range(2):
                tmp = small.tile([C, 1], fp32, tag=f"nt{it}", name="tmp")
                nc.vector.tensor_mul(tmp, ycur, ycur)
                nc.vector.tensor_scalar(
                    out=tmp, in0=tmp, scalar1=vpe, scalar2=-0.5,
                    op0=ALU.mult, op1=ALU.mult,
                )
                nc.vector.tensor_scalar(
                    out=tmp, in0=tmp, scalar1=1.5, scalar2=None, op0=ALU.add
                )
                ynew = small.tile([C, 1], fp32, tag=f"ny{it}", name="ynew")
                nc.vector.tensor_mul(ynew, ycur, tmp)
                ycur = ynew
            alpha = small.tile([C, 1], fp32)
            nc.vector.tensor_mul(alpha, ycur, gsc)
            # hn = alpha*x - beta',  beta' = mean*alpha - gbias
            betap = small.tile([C, 1], fp32)
            nc.vector.tensor_scalar(
                out=betap, in0=g2s[:, 0:1], scalar1=alpha, scalar2=gbi,
                op0=ALU.mult, op1=ALU.subtract,
            )
            # HAM keep-warm blip between Rmm and the q2 matmuls
            nc.tensor.matmul(
                warm_ps[:, 0:1], rmat_sb, alpha, start=True, stop=True
            )

            # hn (q half) + q2/q3 per q-block; alpha folds into the copy.
            # q-block 0 first -- it gates the first scores matmul.
            nc.vector.tensor_scalar(
                out=hnq[:, 0:QB], in0=xh[:, 0:QB],
                scalar1=alpha, scalar2=betap, op0=ALU.mult, op1=ALU.subtract,
            )
            ps2_0 = qk_ps.tile([C, 512], fp32, tag="qk", name="ps2")
            nc.tensor.matmul(
                ps2_0, mqk_sb, hnq[:, 0:512], start=True, stop=True
            )
            nc.vector.tensor_scalar_mul(q3_sb[:, 0, :], ps2_0, alpha)
            for j in range(1, NQB):
                nc.vector.tensor_scalar(
                    out=hnq[:, j * QB : (j + 1) * QB],
                    in0=xh[:, j * QB : (j + 1) * QB],
                    scalar1=alpha, scalar2=betap,
                    op0=ALU.mult, op1=ALU.subtract,
                )
                ps2 = qk_ps.tile([C, 512], fp32, tag="qk", name="ps2")
                nc.tensor.matmul(
                    ps2, mqk_sb, hnq[:, j * 512 : (j + 1) * 512],
                    start=True, stop=True,
                )
                nc.vector.tensor_scalar_mul(q3_sb[:, j, :], ps2, alpha)
            # Bmat = ones_col . (beta/alpha)^T
            ralpha = small.tile([C, 1], fp32)
            nc.vector.reciprocal(ralpha, alpha)
            bo_h = small.tile([C, 1], fp16)
            nc.vector.tensor_scalar(
                out=bo_h, in0=betap, scalar1=ralpha, scalar2=-1.0,
                op0=ALU.mult, op1=ALU.mult,
            )
            # Bmat rows all equal bo^T: partition-broadcast via DMA bounce
            # (no PE/DVE work; ready well before the first q-block boundary)
            nc.sync.dma_start(out=bo_dram[0:1, :], in_=bo_h)
            bo_ap = bo_dram[0:1, :]
            nc.sync.dma_start(
                out=bmat_sb,
                in_=bass.AP(
                    tensor=bo_ap.tensor, offset=bo_ap.offset, ap=[[0, C], [1, C]]
                ),
            )
            if with_qbias:
                u2_h = small.tile([C, 1], fp16)
                nc.vector.tensor_scalar_mul(u2_h, uq_f, alpha)
                for j in range(N // 512):
                    gp = qk_ps.tile([1, 512], fp32, tag="qg")
                    nc.tensor.matmul(
                        gp, u2_h, xh[:, j * 512 : (j + 1) * 512],
                        start=True, stop=True,
                    )
                    nc.vector.tensor_copy(
                        gam_sb[:, j * 512 : (j + 1) * 512], gp
                    )

        # ---- attention main loop: flat software pipeline over (qb, group) ----
        g_kt0 = []
        kt = 0
        for gs in KT_GROUPS:
            g_kt0.append(kt)
            kt += gs

        with (
            tc.tile_pool(name="s_ps", bufs=2, space="PSUM") as spool,
            tc.tile_pool(name="o_ps", bufs=1, space="PSUM") as opool,
            tc.tile_pool(name="bd_ps", bufs=1, space="PSUM") as bdpool,
            tc.tile_pool(name="attn", bufs=6) as apool,
            tc.tile_pool(name="dacc", bufs=2) as dpool,
        ):
            state = {}
            pending_y = None

            def emit_y(pend):
                pp_sb_, rbc_, qb_ = pend
                y_sb = small.tile([C, QB], fp32, tag="y", bufs=2)
                nc.vector.tensor_mul(y_sb, pp_sb_, rbc_)
                nc.gpsimd.tensor_add(
                    y_sb, y_sb, xpb_sb[:, qb_ * QB : (qb_ + 1) * QB]
                )
                if qb_ == NQB - 1:
                    nc.sync.dma_start(
                        out=y_d[:, qb_ * QB : qb_ * QB + 256],
                        in_=y_sb[:, 0:256],
                    )
                    nc.scalar.dma_start(
                        out=y_d[:, qb_ * QB + 256 : (qb_ + 1) * QB],
                        in_=y_sb[:, 256:512],
                    )
                else:
                    nc.sync.dma_start(
                        out=y_d[:, qb_ * QB : (qb_ + 1) * QB], in_=y_sb
                    )

            def emit_scores(qb, g):
                gsize = KT_GROUPS[g]
                kt0 = g_kt0[g]
                s_ps = spool.tile([KT, 3, QB], fp32, tag="s", name="s_ps")
                qv = q3_sb[:, qb, :]
                for t in range(gsize):
                    nc.tensor.matmul(
                        s_ps[:, t, :],
                        xh[:, (kt0 + t) * KT : (kt0 + t + 1) * KT],
                        qv,
                        start=True,
                        stop=(not with_qbias),
                    )
                    if with_qbias:
                        nc.tensor.matmul(
                            s_ps[:, t, :],
                            gam_sb[:, (kt0 + t) * KT : (kt0 + t + 1) * KT],
                            ones_row_h,
                            start=False,
                            stop=True,
                        )
                at = apool.tile([KT, 3, QB], fp16, tag="at", name="at")
                nc.scalar.activation(at[:, :gsize, :], s_ps[:, :gsize, :], AF.Exp)
                return at

            def emit_post(qb, g, at):
                gsize = KT_GROUPS[g]
                kt0 = g_kt0[g]
                st = state[qb]
                for t in range(gsize):
                    k_idx = kt0 + t
                    nc.tensor.matmul(
                        st["out_ps"],
                        xT_sb[:, k_idx, :],
                        at[:, t, :],
                        start=(k_idx == 0),
                        stop=False,
                    )
                # den accumulation; the initial copy rides the idle GpSimd
                # (done before the g==1 DVE add needs it).  The last q-block
                # skips the final DVE add: its 2 tiles fold directly on the
                # PE at the boundary (shortens the output tail).
                if g == 0:
                    nc.gpsimd.tensor_copy(st["dacc"], at)
                elif g == NG - 1:
                    if qb != NQB - 1:
                        nc.vector.tensor_add(
                            st["dacc"][:, 0:2, :], st["dacc"][:, 0:2, :],
                            at[:, 0:2, :],
                        )
                else:
                    nc.vector.tensor_add(st["dacc"], st["dacc"], at)

            def emit_boundary(qb, at_last):
                st = state[qb]
                last = qb == NQB - 1
                # beta*den correction closes the V accumulation group; alpha
                # folds into the copy-out.  For the last q-block the final
                # group's tiles enter directly (dacc stops at group 9).
                extra = (
                    [at_last[:, t, :] for t in range(KT_GROUPS[NG - 1])]
                    if last else []
                )
                dparts = [st["dacc"][:, s, :] for s in range(3)] + extra
                for s, dp in enumerate(dparts):
                    nc.tensor.matmul(
                        st["out_ps"], bmat_sb, dp,
                        start=False, stop=(s == len(dparts) - 1),
                    )
                nc.vector.tensor_scalar_mul(
                    aout_sb[:, qb, :], st["out_ps"], alpha
                )
                # den fold -> 1/den -> PE partition-broadcast (PSUM-resident)
                bd_t = bdpool.tile([C, QB], fp32, tag="bd", name="bd_t")
                for s, dp in enumerate(dparts):
                    nc.tensor.matmul(
                        bd_t[0:1, 0:QB], ones_col_h, dp,
                        start=(s == 0), stop=(s == len(dparts) - 1),
                    )
                rden = small.tile([1, QB], f32r, tag="rden", bufs=2)
                with nc.allow_low_precision(
                    reason="f32r is full fp32 bits; only the PE mode is relaxed"
                ):
                    nc.vector.reciprocal(rden, bd_t[0:1, 0:QB])
                nc.tensor.matmul(bd_t, ones_rowc_r, rden, start=True, stop=True)
                # projection reuses the (just copied-out) V-accumulator bank
                pp = opool.tile([C, QB], fp32, tag="o", name="pp_t")
                nc.tensor.matmul(pp, w2t_sb, aout_sb[:, qb, :], start=True, stop=True)
                pp_sb = small.tile([C, QB], fp32, tag="ppsb", bufs=2)
                nc.vector.tensor_copy(pp_sb, pp)
                return (pp_sb, bd_t, qb)

            seq = [(qb, g) for qb in range(NQB) for g in range(NG)]
            prev = None
            for (qb, g) in seq:
                if g == 0:
                    state[qb] = {
                        "out_ps": opool.tile(
                            [C, QB], fp32, tag="o", name="out_ps"
                        ),
                        "dacc": dpool.tile(
                            [KT, 3, QB], fp16, tag="d", name="dacc"
                        ),
                    }
                at = emit_scores(qb, g)
                if prev is not None:
                    pqb, pg, pat = prev
                    emit_post(pqb, pg, pat)
                    if pg == NG - 1:
                        pending_y = emit_boundary(pqb, pat)
                if g == 2 and pending_y is not None:
                    emit_y(pending_y)
                    pending_y = None
                prev = (qb, g, at)

            pqb, pg, pat = prev
            emit_post(pqb, pg, pat)
            pending_y_last = emit_boundary(pqb, pat)
            if pending_y is not None:
                emit_y(pending_y)
            emit_y(pending_y_last)

        if rep_ctx is not None:
            rep_ctx.__exit__(None, None, None)

    nc.compile()
    return nc


def _prep_maps(x):
    x = np.ascontiguousarray(np.asarray(x, dtype=np.float32))
    b, c, h, w = x.shape
    assert (b, c, h * w) == (B, C, N), f"unexpected shape {x.shape}"
    return x.reshape(b, c, h * w)


def _make_in_maps(x, norm_scale, norm_bias, wq, bq, wk, bk, wv, bv, wp, bp):
    xr = _prep_maps(x)
    s = float(C) ** -0.5
    f32 = np.float32
    f64 = np.float64

    wqs = np.asarray(wq, f64) * s
    wk64 = np.asarray(wk, f64)
    wv64 = np.asarray(wv, f64)
    wp64 = np.asarray(wp, f64)
    bq64 = np.asarray(bq, f64) * s
    bv64 = np.asarray(bv, f64)
    bp64 = np.asarray(bp, f64)

    # scores: hn^T (Wk^T Wq') hn ; lhsT for q2 = M.hn is M^T = Wq'^T Wk
    mqk = (wqs.T @ wk64).astype(np.float16)
    # proj: W2 = Wp.Wv, lhsT = W2^T ; bias bp2 = bp + Wp.bv (folded into xpb)
    w2t = (wp64 @ wv64).T.astype(f32)
    bp2 = (bp64 + wp64 @ bv64).astype(f32).reshape(C, 1)
    # q-bias term (slow path only): u = Wk^T bq'
    uq = (wk64.T @ bq64).astype(f32).reshape(C, 1)
    ident = np.eye(C, dtype=np.float16)
    gsc = np.asarray(norm_scale, f32).reshape(C, 1)
    gbi = np.asarray(norm_bias, f32).reshape(C, 1)
    gsz = C // GROUPS
    grp = np.arange(C) // gsz
    rmat = (grp[:, None] == grp[None, :]).astype(f32) / gsz

    pk32 = np.ascontiguousarray(
        np.concatenate([gsc, gbi, uq, rmat, w2t], axis=1)
    )
    assert pk32.shape == (C, NPK32)
    pk16 = np.ascontiguousarray(np.concatenate([mqk, ident], axis=1))

    with_qbias = bool(np.any(np.asarray(bq) != 0))

    in_maps = []
    for core in range(NCORES):
        bi, hi = core // 2, core % 2
        xb = xr[bi]
        if hi:
            xb = np.roll(xb, -NQ, axis=1)
        in_maps.append(
            dict(
                xh=np.ascontiguousarray(xb.astype(np.float16)),
                xpb=np.ascontiguousarray(xb[:, :NQ] + bp2),
                pk32=pk32, pk16=pk16,
            )
        )
    return in_maps, with_qbias


def kernel(x, norm_scale, norm_bias, wq, bq, wk, bk, wv, bv, wp, bp):
    from concourse.bass_utils import run_bass_kernel_spmd

    in_maps, with_qbias = _make_in_maps(
        x, norm_scale, norm_bias, wq, bq, wk, bk, wv, bv, wp, bp
    )

    key = ("nc", with_qbias)
    if key not in _CACHE:
        _CACHE[key] = _build_nc(with_qbias=with_qbias)
    res = run_bass_kernel_spmd(
        _CACHE[key], in_maps, core_ids=list(range(NCORES)), **_CACHE.get("runkw", {})
    )
    _CACHE["last_result"] = res

    out = np.empty((B, C, N), np.float32)
    for core in range(NCORES):
        bi, hi = core // 2, core % 2
        out[bi, :, hi * NQ : (hi + 1) * NQ] = res.results[core]["y"]
    return out.reshape(B, C, 64, 64)
